# revision 3
# baseline (speedup 1.0000x reference)
"""Trainium2 Bass kernel for 3D-conv attention (4 heads x dim 32, N=4096).

Sharding: one (batch, head) pair per NeuronCore (2 batches x 4 heads = 8 cores).
The tiny projections q = (scale*Wq_h)@x, k = Wk_h@x, v = Wv_h@x run on the
HOST in fp32 and ship as fp16 (q/k 4x-replicated over partitions) plus an
fp8e4 DoubleRow-packed v^T with a ones column. Each core computes:
    S^T = k.T @ q                  (keys j on partitions, queries i free)
    E   = exp8(S^T - SHIFT)        (fp8e4; shift makes max fit under 240)
    [O_unnorm; s] = vT_dr.T @ E    (fp8 DoubleRow matmul, ones col rides M=33)
Host: out[b] = sum_h Wo_h.T @ (O_unnorm_h / s_h) + b_out  (fp32 projection).

Perf structure (v2 -- exp-roofline driven):
- Only ACT and DVE can read PSUM on TRN2 (Pool is rejected by the BIR
  verifier), so the N^2 exp stream is the hard floor: every score element
  costs one ACT-or-DVE lane-cycle (fp32 PSUM input forbids all DVE 2x
  modes). The schedule exists to keep those two engines saturated with exp
  and nothing else.
- E is fp8e4 so PV runs in DoubleRow perf mode (2 fp8 MACs/PE-cell/cycle):
  one [33,512] accumulation chain per i-tile, two DR matmuls per 4-chunk
  iteration (contraction 256 = 2 chunks each). PV column-streaming halves
  vs fp16, PE drops well below the exp cadence and off the critical path.
- DVE chunks: Schraudolph fp8 bit trick in ONE tensor_scalar:
  uint8(out) = round(8*log2(e)*S + B). The fp32->uint8 write rounds AND
  saturates at 0 (verified on HW), which implements the underflow clamp for
  free; B keeps the max at ~78 < 127 so the sign bit never sets.
- ACT chunks: native Exp with bias=-SHIFT writing fp8e4 directly (RNE on
  the conversion, verified) -- true exp, so ACT chunks are more accurate
  than DVE chunks.
- Score PSUM: a 2-bank DVE pair tile (bufs=1) + 2x2-bank ACT pair tiles +
  2 PV output banks = 8 banks exactly. Every P13 rebalance iteration ACT
  additionally takes the DVE tile's slot-1 chunk (emitted BEFORE the ACT
  pair so the bank frees early).
- The per-tile epilogue is one ACT copy [33,512] PSUM->SBUF fp16 + DMA
  (the old 2-band add died with the single DR chain), emitted one iteration
  after the tile's last lagged PV, so the PE stream has no tile-boundary
  stall and the HAM p-state stays warm.
"""

import numpy as np
import ml_dtypes

import concourse.bass as bass
import concourse.tile as tile
from concourse import bacc, mybir
from concourse.bass_utils import run_bass_kernel_spmd

HEADS = 4
DH = 32
DIM = 128
N = 4096
TI = 512            # i-tile (query) width = one PSUM bank of fp32
NT = N // TI        # 8 i-tiles
CH = 128            # j-chunk width = PE partition count
NCH = N // CH       # 32 chunks
NPAIR = NCH // 2    # 16 chunk-pairs

F32 = mybir.dt.float32
F16 = mybir.dt.float16
F8E4 = mybir.dt.float8e4
U8 = mybir.dt.uint8
EXP = mybir.ActivationFunctionType.Exp
DR = mybir.MatmulPerfMode.DoubleRow

# exp shift: softmax-invariant; keeps e^(S-SHIFT) <= ~104 < 240 (fp8e4 max)
SHIFT = 2.75
# Schraudolph fp8e4 bit-trick constants (DVE chunks)
EXP_A8 = 8.0 / float(np.log(2.0))
EXP_B8 = 56.0 - EXP_A8 * SHIFT

# every REB_EVERY-th iteration ACT takes 3 of the 4 chunks (P13 pattern)
REB_EVERY = 4

N_CORES = 8

LAST_RESULTS = None  # BassKernelResults of the most recent run (for harness)
TRACE = False


def _ensure_ntff_hook():
    """Make ``antenv.axon_hooks`` importable so trace-enabled runs work (or
    degrade gracefully). Profiling only; correctness never depends on it."""
    try:
        import antenv.axon_hooks  # noqa: F401
        return True
    except ImportError:
        pass
    import sys
    import types
    hook = None
    try:
        from trn_agent_boot.trn_boot import _ntff_profile_via_ctypes
        hook = _ntff_profile_via_ctypes("/opt/axon/libaxon_pjrt.so")
    except Exception:
        pass
    try:
        import antenv
        mod = types.ModuleType("antenv.axon_hooks")
        state = {"hook": hook}
        mod.get_axon_ntff_profile_hook = lambda: state["hook"]
        mod.set_axon_ntff_profile_hook = lambda h: state.update(hook=h)
        sys.modules["antenv.axon_hooks"] = mod
        antenv.axon_hooks = mod
    except Exception as e:  # pragma: no cover
        print(f"ntff hook setup failed ({e}); running without trace")
        return False
    return hook is not None


def build_nc():
    nc = bacc.Bacc(None)
    q_d = nc.dram_tensor("q4", [4 * DH, N], F16, kind="ExternalInput")
    k_d = nc.dram_tensor("k4", [4 * DH, N], F16, kind="ExternalInput")
    vt_d = nc.dram_tensor("vTo", [DIM, NPAIR, 2, 64], F8E4, kind="ExternalInput")
    o_d = nc.dram_tensor("o", [DH + 1, N], F16, kind="ExternalOutput")

    with tile.TileContext(nc) as tc:
        with (
            tc.tile_pool(name="singles", bufs=1) as singles,
            tc.tile_pool(name="ep", bufs=6) as ep,
            tc.tile_pool(name="outp", bufs=3) as outp,
            tc.tile_pool(name="psS", bufs=6, space="PSUM") as psS,
            tc.tile_pool(name="psO", bufs=2, space="PSUM") as psO,
        ):
            q_sb = singles.tile([4 * DH, N], F16)
            k_sb = singles.tile([4 * DH, N], F16)
            vT = singles.tile([DIM, NPAIR, 2, 64], F8E4)
            nbias = singles.tile([DIM, 1], F32)
            nc.vector.memset(nbias[:], -SHIFT)
            # k fully first (i-tile 0 needs every chunk), then q tile 0,
            # then v^T (needed LAG chunks into the main loop), then the rest.
            nc.sync.dma_start(out=k_sb[:, 0:TI], in_=k_d[:, 0:TI])
            nc.sync.dma_start(out=q_sb[:, 0:TI], in_=q_d[:, 0:TI])
            nc.sync.dma_start(out=k_sb[:, TI : N // 2], in_=k_d[:, TI : N // 2])
            nc.sync.dma_start(out=k_sb[:, N // 2 : N], in_=k_d[:, N // 2 : N])
            nc.sync.dma_start(out=vT[:], in_=vt_d[:])
            nc.sync.dma_start(out=q_sb[:, TI:N], in_=q_d[:, TI:N])

            def emit_epilogue(t, pO):
                if t == NT - 1:
                    # pure tail: two half-width copies so the first half's
                    # DMA runs under the second half's copy.
                    for h in range(2):
                        sl = slice(h * (TI // 2), (h + 1) * (TI // 2))
                        os_h = outp.tile([DH + 1, TI // 2], F16,
                                         tag="osl%d" % h)
                        nc.scalar.copy(os_h[:], pO[0 : DH + 1, sl])
                        lo = t * TI + h * (TI // 2)
                        nc.sync.dma_start(out=o_d[:, lo : lo + TI // 2],
                                          in_=os_h[:])
                    return
                os33 = outp.tile([DH + 1, TI], F16, tag="os")
                nc.scalar.copy(os33[:], pO[0 : DH + 1, :])
                nc.sync.dma_start(out=o_d[:, bass.ts(t, TI)], in_=os33[:])

            # Continuous software pipeline over all NT*NCH chunks: the PV
            # stream lags the S^T/exp stream by LAG chunks GLOBALLY, so
            # every iteration issues [2 DR PVs][4 S^Ts][exps] -- uniform PE
            # duty and uniform exp load, with no tile-boundary bunching.
            LAG = 12
            pOs = {}   # tile -> PSUM accumulator [33, TI]
            ess = {}   # tile -> list of per-iteration E tiles [128, 4, TI]

            def pv_mms(tp, cp):
                pO = pOs[tp]
                e4 = ess[tp][cp // 4]
                pp = cp // 2
                for j in range(2):
                    nc.tensor.matmul(
                        pO[:], vT[:, pp + j, :, 0 : DH + 1],
                        e4[:, 2 * j : 2 * j + 2, :],
                        start=(cp == 0 and j == 0),
                        stop=(cp == NCH - 4 and j == 1),
                        perf_mode=DR, skip_group_check=True)

            def st_mm(pS_ap, t, c):
                b = 32 * (c % 4)
                nc.tensor.matmul(pS_ap,
                                 k_sb[bass.ds(b, DH), bass.ts(c, CH)],
                                 q_sb[bass.ds(b, DH), bass.ts(t, TI)],
                                 start=True, stop=True, tile_position=(b, 0))

            NG = NT * NCH
            ep_due = None  # delayed one iteration so the last PVs complete
            it = 0
            for g in range(0, NG + LAG + 4, 4):
                if ep_due is not None:
                    emit_epilogue(ep_due, pOs.pop(ep_due))
                    ep_due = None
                # lagged PV stream (chunks g-LAG .. g-LAG+3)
                gp = g - LAG
                if gp >= 0 and gp < NG:
                    tp, cp = divmod(gp, NCH)
                    pv_mms(tp, cp)
                    if cp + 4 == NCH:
                        ep_due = tp
                if g >= NG:
                    continue
                # S^T + exp stream (chunks g .. g+3 of tile ti)
                ti, c0 = divmod(g, NCH)
                if c0 == 0:
                    pO_t = psO.tile([DH + 1, TI], F32, tag="po")
                    pOs[ti] = pO_t
                    ess[ti] = [None] * (NCH // 4)
                # PSUM: DVE pair (bufs=1 ring) + ACT pair (bufs=2 ring);
                # the four S^T matmuls cover all four 32-row PE bands.
                pS_d = psS.tile([DIM, 2, TI], F32, tag="psd", bufs=1)
                pS_a = psS.tile([DIM, 2, TI], F32, tag="psa", bufs=2)
                st_mm(pS_d[:, 0, :], ti, c0)
                st_mm(pS_d[:, 1, :], ti, c0 + 1)
                st_mm(pS_a[:, 0, :], ti, c0 + 2)
                st_mm(pS_a[:, 1, :], ti, c0 + 3)
                e4 = ep.tile([DIM, 4, TI], F8E4, tag="e4")
                ess[ti][c0 // 4] = e4
                p13 = (it % REB_EVERY == REB_EVERY - 1)
                if p13:
                    # ACT takes the DVE tile's slot-1 chunk. Emit it FIRST
                    # so the shared DVE psum bank frees early.
                    nc.scalar.activation(e4[:, 1, :], pS_d[:, 1, :],
                                         func=EXP, bias=nbias[:])
                    nc.vector.tensor_scalar(
                        out=e4[:, 0, :].bitcast(U8), in0=pS_d[:, 0, :],
                        scalar1=EXP_A8, scalar2=EXP_B8,
                        op0=mybir.AluOpType.mult, op1=mybir.AluOpType.add)
                else:
                    nc.vector.tensor_scalar(
                        out=e4[:, 0:2, :].bitcast(U8), in0=pS_d[:],
                        scalar1=EXP_A8, scalar2=EXP_B8,
                        op0=mybir.AluOpType.mult, op1=mybir.AluOpType.add)
                nc.scalar.activation(e4[:, 2:4, :], pS_a[:],
                                     func=EXP, bias=nbias[:])
                it += 1
                if g < LAG:
                    # PE warmup: pad the pipeline-fill head with dummy
                    # matmuls into a region the start=True PV chain fully
                    # overwrites later, so the HAM clock locks warm.
                    for _ in range(3):
                        nc.tensor.matmul(pOs[0][:],
                                         k_sb[0:DH, 0 : DH + 1],
                                         q_sb[0:DH, 0:TI], start=True,
                                         stop=True, skip_group_check=True)
    nc.compile()
    return nc


def kernel(input, w_qkv, w_out, b_out):
    global LAST_RESULTS
    input = np.asarray(input, dtype=np.float32)
    w_qkv = np.asarray(w_qkv, dtype=np.float32)
    w_out = np.asarray(w_out, dtype=np.float32)
    b_out = np.asarray(b_out, dtype=np.float32)

    b, c, X, Y, Z = input.shape
    n = X * Y * Z
    assert (b, c, n) == (2, DIM, N), (b, c, n)
    xf = input.reshape(b, c, n)
    scale = DH ** -0.5
    hid = HEADS * DH

    in_maps = []
    for core in range(N_CORES):
        bi, h = divmod(core, HEADS)
        wq = w_qkv[h * DH : (h + 1) * DH, :] * scale
        wk = w_qkv[hid + h * DH : hid + (h + 1) * DH, :]
        wv = w_qkv[2 * hid + h * DH : 2 * hid + (h + 1) * DH, :]
        xb = xf[bi]
        q = (wq @ xb).astype(np.float16)            # [32, N]
        k = (wk @ xb).astype(np.float16)
        v = (wv @ xb).astype(np.float32)            # [32, N]
        # DoubleRow-packed v^T: [DIM, NPAIR, 2, 64] fp8e4 with ones col 32
        vt = np.zeros((DIM, NPAIR, 2, 64), np.float32)
        vt[:, :, :, DH] = 1.0
        # vt[p, pp, s, d] = v[d, (2*pp+s)*128 + p]
        vt[:, :, :, 0:DH] = v.T.reshape(NPAIR, 2, CH, DH).transpose(2, 0, 1, 3)
        vt8 = vt.astype(ml_dtypes.float8_e4m3)
        in_maps.append({
            "q4": np.ascontiguousarray(np.tile(q, (4, 1))),
            "k4": np.ascontiguousarray(np.tile(k, (4, 1))),
            "vTo": vt8,
        })

    nc = build_nc()
    hook_ok = _ensure_ntff_hook()  # also guards env-driven BASS_TRACE runs
    LAST_RESULTS = run_bass_kernel_spmd(nc, in_maps, list(range(N_CORES)),
                                        trace=TRACE and hook_ok)
    results = LAST_RESULTS.results

    out = np.zeros((b, c, n), np.float32)
    for core in range(N_CORES):
        bi, h = divmod(core, HEADS)
        o33 = results[core]["o"].astype(np.float32)
        attn = o33[0:DH] / o33[DH : DH + 1]
        out[bi] += w_out[:, h * DH : (h + 1) * DH] @ attn
    out += b_out[None, :, None]
    return out.reshape(b, c, X, Y, Z)


# revision 4
# speedup vs baseline: 1.0084x; 1.0084x over previous
"""Trainium2 Bass kernel for 3D-conv attention (4 heads x dim 32, N=4096).

Sharding: one (batch, head) pair per NeuronCore (2 batches x 4 heads = 8 cores).
The tiny projections q = (scale*Wq_h)@x, k = Wk_h@x, v = Wv_h@x run on the
HOST in fp32 and ship as fp16 (q/k 4x-replicated over partitions, v^T in
per-chunk [128, 33] blocks with a ones column). Each core computes:
    S^T = k.T @ q               (keys j on partitions, queries i free)
    E = exp(S^T)                (fp16; no max subtraction: |S| < ~7.4)
    [O_unnorm; s] = [vT | 1].T @ E   (ones column rides the row-sum in M=33)
Host: out[b] = sum_h Wo_h.T @ (O_unnorm_h / s_h) + b_out  (fp32 projection).

Perf structure (v3 -- measured-roofline driven):
- Only ACT and DVE can read PSUM on TRN2 (Pool is rejected by the BIR
  verifier), so the N^2 exp stream is the hard floor: every score element
  costs one ACT-or-DVE lane-cycle (fp32 PSUM input forbids all DVE 2x
  modes). ~256 [128,512] chunks/core at DVE ~598ns / ACT ~502ns a chunk.
- Microbench facts (probe_dr): a dependency-free PE stream runs 216ns per
  512-col matmul (1 col/cycle @ 2.4GHz) and 4-way row-banded S^T quads
  truly overlap (213ns for 4 chunks) -- but the PE p-state starts at
  1.2GHz and only ramps to 2.4GHz after ~3us of CONTINUOUS execution; any
  stall re-throttles it (fp8 DoubleRow measured 0 gain: moving-fetch is
  byte-bandwidth-limited, so fp16 everywhere).
- So the schedule keeps the PE stream dense and just below the exp
  cadence: per iteration [4 serial PV chain matmuls (~864ns)] [pads]
  [4-way S^T quad (~213ns)]. Pads are dummy matmuls into unread PSUM rows
  64-95 of the pO bank, placed BEFORE the quad (PE executes in order; the
  quad is the only instruction that can block on score-bank frees).
- PV is ONE serial [33,512] accumulation chain per i-tile (no 2-band
  column tiling): the serial chain streams at full rate back-to-back, and
  the single band kills the old band-add epilogue -- the epilogue is one
  ACT copy [33,512] PSUM->SBUF fp16 + DMA, emitted one iteration after
  the tile's last lagged PV, so the PE has no tile-boundary stall.
- exp split: DVE takes chunks 0-1 as ONE [128,2,512] int16-bitcast
  Schraudolph op; ACT takes chunks 2-3 as one native-Exp pair. Every
  REB_EVERY-th iteration ACT also takes chunk 1 (DVE 598ns/chunk vs ACT
  ~502: optimum is ~1.8/2.2) -- the ACT single is emitted FIRST so the
  shared DVE psum bank frees early.
- Score PSUM: DVE pair tile (2 banks, bufs=1) + ACT pair (2x2 banks) +
  pO [128,512] x2 (rows 0-32 real, 64-95 pad scratch) = 8 banks exactly.
"""

import numpy as np

import concourse.bass as bass
import concourse.tile as tile
from concourse import bacc, mybir
from concourse.bass_utils import run_bass_kernel_spmd

HEADS = 4
DH = 32
DIM = 128
N = 4096
TI = 512            # i-tile (query) width = one PSUM bank of fp32
NT = N // TI        # 8 i-tiles
CH = 128            # j-chunk width = PE partition count
NCH = N // CH       # 32 chunks

F32 = mybir.dt.float32
F16 = mybir.dt.float16
I16 = mybir.dt.int16
EXP = mybir.ActivationFunctionType.Exp

# Schraudolph fp16 bit-trick constants (DVE chunks); B tuned on real data.
EXP_A = 1024.0 / float(np.log(2.0))
EXP_B = 15360.0 - 60.0

REB_EVERY = 4   # every REB_EVERY-th iteration ACT takes 3 of the 4 chunks
PAD_COLS = 256  # pad matmul width (dummy PE work to hold the 2.4GHz p-state)
PADS_PER_ITER = 1

N_CORES = 8
_np_f16 = np.float16

LAST_RESULTS = None  # BassKernelResults of the most recent run (for harness)
TRACE = False


def _ensure_ntff_hook():
    """Make ``antenv.axon_hooks`` importable so trace-enabled runs work (or
    degrade gracefully). Profiling only; correctness never depends on it."""
    try:
        import antenv.axon_hooks  # noqa: F401
        return True
    except ImportError:
        pass
    import sys
    import types
    hook = None
    try:
        from trn_agent_boot.trn_boot import _ntff_profile_via_ctypes
        hook = _ntff_profile_via_ctypes("/opt/axon/libaxon_pjrt.so")
    except Exception:
        pass
    try:
        import antenv
        mod = types.ModuleType("antenv.axon_hooks")
        state = {"hook": hook}
        mod.get_axon_ntff_profile_hook = lambda: state["hook"]
        mod.set_axon_ntff_profile_hook = lambda h: state.update(hook=h)
        sys.modules["antenv.axon_hooks"] = mod
        antenv.axon_hooks = mod
    except Exception as e:  # pragma: no cover
        print(f"ntff hook setup failed ({e}); running without trace")
        return False
    return hook is not None


def build_nc():
    nc = bacc.Bacc(None)
    q_d = nc.dram_tensor("q4", [4 * DH, N], F16, kind="ExternalInput")
    k_d = nc.dram_tensor("k4", [4 * DH, N], F16, kind="ExternalInput")
    vt_d = nc.dram_tensor("vTo", [DIM, NCH, DH + 1], F16, kind="ExternalInput")
    o_d = nc.dram_tensor("o", [DH + 1, N], F16, kind="ExternalOutput")

    with tile.TileContext(nc) as tc:
        with (
            tc.tile_pool(name="singles", bufs=1) as singles,
            tc.tile_pool(name="ep", bufs=6) as ep,
            tc.tile_pool(name="outp", bufs=3) as outp,
            tc.tile_pool(name="psS", bufs=6, space="PSUM") as psS,
            tc.tile_pool(name="psO", bufs=2, space="PSUM") as psO,
        ):
            q_sb = singles.tile([4 * DH, N], F16)
            k_sb = singles.tile([4 * DH, N], F16)
            vT = singles.tile([DIM, NCH, DH + 1], F16)
            # k fully first (i-tile 0 needs every chunk), then q tile 0,
            # then v^T (needed LAG chunks into the main loop), then the rest.
            nc.sync.dma_start(out=k_sb[:, 0:TI], in_=k_d[:, 0:TI])
            nc.sync.dma_start(out=q_sb[:, 0:TI], in_=q_d[:, 0:TI])
            nc.sync.dma_start(out=k_sb[:, TI : N // 2], in_=k_d[:, TI : N // 2])
            nc.sync.dma_start(out=k_sb[:, N // 2 : N], in_=k_d[:, N // 2 : N])
            nc.sync.dma_start(out=vT[:], in_=vt_d[:])
            nc.sync.dma_start(out=q_sb[:, TI:N], in_=q_d[:, TI:N])

            def emit_epilogue(t, pO):
                if t == NT - 1:
                    # pure tail: two half-width copies so the first half's
                    # DMA runs under the second half's copy.
                    for h in range(2):
                        sl = slice(h * (TI // 2), (h + 1) * (TI // 2))
                        os_h = outp.tile([DH + 1, TI // 2], F16,
                                         tag="osl%d" % h)
                        nc.scalar.copy(os_h[:], pO[0 : DH + 1, sl])
                        lo = t * TI + h * (TI // 2)
                        nc.sync.dma_start(out=o_d[:, lo : lo + TI // 2],
                                          in_=os_h[:])
                    return
                os33 = outp.tile([DH + 1, TI], F16, tag="os")
                nc.scalar.copy(os33[:], pO[0 : DH + 1, :])
                nc.sync.dma_start(out=o_d[:, bass.ts(t, TI)], in_=os33[:])

            # Continuous software pipeline over all NT*NCH chunks: the PV
            # stream lags the S^T/exp stream by LAG chunks GLOBALLY, so
            # every iteration issues [4 PVs][pads][4 S^Ts][exps].
            LAG = 12
            pOs = {}   # tile -> PSUM accumulator [128, TI] (rows 0-32 real)
            ess = {}   # tile -> list of per-iteration E tiles [128, 4, TI]

            def pv_mms(tp, cp):
                pO = pOs[tp]
                e4 = ess[tp][cp // 4]
                for j in range(4):
                    c = cp + j
                    nc.tensor.matmul(
                        pO[0 : DH + 1, :], vT[:, c, :], e4[:, j, :],
                        start=(c == 0), stop=(c == NCH - 1),
                        skip_group_check=True)

            def st_mm(pS_ap, t, c):
                b = 32 * (c % 4)
                nc.tensor.matmul(pS_ap,
                                 k_sb[bass.ds(b, DH), bass.ts(c, CH)],
                                 q_sb[bass.ds(b, DH), bass.ts(t, TI)],
                                 start=True, stop=True, tile_position=(b, 0))

            def pad_mm(pO, ncols):
                # dummy matmul into rows 64-95 of the pO bank (never read;
                # aligned col tile (.,64)); holds the PE p-state warm.
                nc.tensor.matmul(pO[bass.ds(64, 32), 0:ncols],
                                 k_sb[0:DH, 0:DH],
                                 q_sb[0:DH, 0:ncols], start=True,
                                 stop=True, skip_group_check=True)

            NG = NT * NCH
            ep_due = None  # delayed one iteration so the last PVs complete
            it = 0
            cur_pO = None  # pO of the tile currently being SCORED (pads)
            for g in range(0, NG + LAG + 4, 4):
                if ep_due is not None:
                    emit_epilogue(ep_due, pOs.pop(ep_due))
                    ep_due = None
                # lagged PV stream (chunks g-LAG .. g-LAG+3)
                gp = g - LAG
                if gp >= 0 and gp < NG:
                    tp, cp = divmod(gp, NCH)
                    pv_mms(tp, cp)
                    if cp + 4 == NCH:
                        ep_due = tp
                if g >= NG:
                    continue
                # S^T + exp stream (chunks g .. g+3 of tile ti)
                ti, c0 = divmod(g, NCH)
                if c0 == 0:
                    pO_t = psO.tile([DIM, TI], F32, tag="po")
                    pOs[ti] = pO_t
                    ess[ti] = [None] * (NCH // 4)
                    cur_pO = pO_t
                # pads sit BEFORE the quad: they execute while the quad's
                # score banks drain, instead of the PE idling there.
                for _ in range(PADS_PER_ITER if g >= LAG else 3):
                    pad_mm(cur_pO, PAD_COLS if g >= LAG else TI)
                # PSUM: DVE pair (bufs=1 ring) + ACT pair (bufs=2 ring);
                # the four S^T matmuls cover all four 32-row PE bands.
                pS_d = psS.tile([DIM, 2, TI], F32, tag="psd", bufs=1)
                pS_a = psS.tile([DIM, 2, TI], F32, tag="psa", bufs=2)
                st_mm(pS_d[:, 0, :], ti, c0)
                st_mm(pS_d[:, 1, :], ti, c0 + 1)
                st_mm(pS_a[:, 0, :], ti, c0 + 2)
                st_mm(pS_a[:, 1, :], ti, c0 + 3)
                e4 = ep.tile([DIM, 4, TI], F16, tag="e4")
                ess[ti][c0 // 4] = e4
                p13 = (it % REB_EVERY == REB_EVERY - 1)
                if p13:
                    # ACT takes the DVE tile's slot-1 chunk; emitted FIRST
                    # so the shared (bufs=1) DVE psum bank frees early.
                    nc.scalar.activation(e4[:, 1, :], pS_d[:, 1, :],
                                         func=EXP)
                    nc.vector.tensor_scalar(
                        out=e4[:, 0, :].bitcast(I16), in0=pS_d[:, 0, :],
                        scalar1=EXP_A, scalar2=EXP_B,
                        op0=mybir.AluOpType.mult, op1=mybir.AluOpType.add)
                else:
                    nc.vector.tensor_scalar(
                        out=e4[:, 0:2, :].bitcast(I16), in0=pS_d[:],
                        scalar1=EXP_A, scalar2=EXP_B,
                        op0=mybir.AluOpType.mult, op1=mybir.AluOpType.add)
                nc.scalar.activation(e4[:, 2:4, :], pS_a[:], func=EXP)
                it += 1
    nc.compile()
    return nc


def kernel(input, w_qkv, w_out, b_out):
    global LAST_RESULTS
    input = np.asarray(input, dtype=np.float32)
    w_qkv = np.asarray(w_qkv, dtype=np.float32)
    w_out = np.asarray(w_out, dtype=np.float32)
    b_out = np.asarray(b_out, dtype=np.float32)

    b, c, X, Y, Z = input.shape
    n = X * Y * Z
    assert (b, c, n) == (2, DIM, N), (b, c, n)
    xf = input.reshape(b, c, n)
    scale = DH ** -0.5
    hid = HEADS * DH

    in_maps = []
    for core in range(N_CORES):
        bi, h = divmod(core, HEADS)
        wq = w_qkv[h * DH : (h + 1) * DH, :] * scale
        wk = w_qkv[hid + h * DH : hid + (h + 1) * DH, :]
        wv = w_qkv[2 * hid + h * DH : 2 * hid + (h + 1) * DH, :]
        xb = xf[bi]
        q = (wq @ xb).astype(_np_f16)            # [32, N]
        k = (wk @ xb).astype(_np_f16)
        v = (wv @ xb).astype(_np_f16)            # [32, N]
        vt = np.empty((DIM, NCH, DH + 1), _np_f16)
        vt[:, :, DH] = 1.0
        vt[:, :, 0:DH] = v.T.reshape(NCH, CH, DH).transpose(1, 0, 2)
        in_maps.append({
            "q4": np.ascontiguousarray(np.tile(q, (4, 1))),
            "k4": np.ascontiguousarray(np.tile(k, (4, 1))),
            "vTo": vt,
        })

    nc = build_nc()
    hook_ok = _ensure_ntff_hook()  # also guards env-driven BASS_TRACE runs
    LAST_RESULTS = run_bass_kernel_spmd(nc, in_maps, list(range(N_CORES)),
                                        trace=TRACE and hook_ok)
    results = LAST_RESULTS.results

    out = np.zeros((b, c, n), np.float32)
    for core in range(N_CORES):
        bi, h = divmod(core, HEADS)
        o33 = results[core]["o"].astype(np.float32)
        attn = o33[0:DH] / o33[DH : DH + 1]
        out[bi] += w_out[:, h * DH : (h + 1) * DH] @ attn
    out += b_out[None, :, None]
    return out.reshape(b, c, X, Y, Z)


# revision 5
# speedup vs baseline: 1.1405x; 1.1310x over previous
"""Trainium2 Bass kernel for 3D-conv attention (4 heads x dim 32, N=4096).

Sharding: one (batch, head) pair per NeuronCore (2 batches x 4 heads = 8 cores).
The tiny projections q = (scale*Wq_h)@x, k = Wk_h@x, v = Wv_h@x run on the
HOST in fp32 and ship as fp16 (q/k 4x-replicated over partitions, v^T in
per-chunk [128, 33] blocks with a ones column). Each core computes:
    S^T = k.T @ q               (keys j on partitions, queries i free)
    E = exp(S^T)                (fp16; no max subtraction: |S| < ~7.4)
    [O_unnorm; s] = [vT | 1].T @ E   (ones column rides the row-sum in M=33)
Host: out[b] = sum_h Wo_h.T @ (O_unnorm_h / s_h) + b_out  (fp32 projection).

Perf structure (v4 -- measured-roofline driven):
- Only ACT and DVE can read PSUM on TRN2 (the BIR verifier rejects Pool),
  so the N^2 exp stream is the hard floor: every score element costs one
  ACT-or-DVE lane-cycle (fp32 PSUM input forbids all DVE 2x perf modes).
- Microbench facts (probe_dr/probe_pe): a dependency-free warm PE does
  4-way row-banded S^T quads in ~300ns and the baseline [2-band PV +
  quad] iteration in 867ns -- BUT the PE p-state starts at 1.2GHz, ramps
  to 2.4GHz only after ~3us of CONTINUOUS execution, and ANY idle gap
  re-throttles it for the next ~3us. fp8 DoubleRow measured ZERO matmul
  gain (moving-fetch is byte-bandwidth-limited), so everything is fp16.
- The exp cadence (~1.05-1.2us/iter) exceeds the warm PE work (~870ns),
  so the PE idles ~200-330ns per iteration at the S^T quad (score banks
  free at exp pace) -- and that idle is what kept the clock at ~1.2-1.6GHz
  historically. Fix: PAD matmuls (256-col dummies into a dedicated spare
  PSUM bank) emitted AFTER each iteration's exps. Priority order makes the
  scheduler pop a pad exactly when the next real matmul isn't ready yet,
  so the PE stays busy through the gap and holds its p-state.
- PSUM: DVE double tile [128,2,512] (bufs=2, 4 banks; the slower exp
  reader gets the deeper ring) + ACT pair (bufs=1, 2 banks) + pO (1 bank;
  the LAG means tile t+1's first PV comes ~3 iterations after tile t's
  epilogue) + 1 spare pad bank = 8 exactly.
- DVE takes chunks 0-1 as ONE [128,2,512] int16-bitcast Schraudolph op;
  ACT takes chunks 2-3 as one native-Exp pair; on c0 in (12,24) ACT also
  takes chunk 1 (DVE 598ns/chunk vs ACT ~510: optimum ~1.75/2.25).
- PV is 2-band column-tiled (bands 0-32 / 64-96 of the pO bank, pairs
  co-stream in the PE), accumulation chains per band; epilogue = ACT band
  copy + DVE add + DMA, software-pipelined one iteration into the next
  tile. The PV stream lags S^T by LAG=12 chunks continuously.
"""

import numpy as np

import concourse.bass as bass
import concourse.tile as tile
from concourse import bacc, mybir
from concourse.bass_utils import run_bass_kernel_spmd

HEADS = 4
DH = 32
DIM = 128
N = 4096
TI = 512            # i-tile (query) width = one PSUM bank of fp32
NT = N // TI        # 8 i-tiles
CH = 128            # j-chunk width = PE partition count
NCH = N // CH       # 32 chunks

F32 = mybir.dt.float32
F16 = mybir.dt.float16
I16 = mybir.dt.int16
EXP = mybir.ActivationFunctionType.Exp

# Schraudolph fp16 bit-trick constants (B tuned on the real data
# distribution; -60 centers the piecewise-linear 2^frac approx error).
EXP_A = 1024.0 / float(np.log(2.0))
EXP_B = 15360.0 - 60.0

PAD_COLS = 256      # pad matmul width
PADS_PER_ITER = 2   # dummy PE work per iteration to hold the 2.4GHz p-state

N_CORES = 8
_np_f16 = np.float16

LAST_RESULTS = None  # BassKernelResults of the most recent run (for harness)
TRACE = False


def _ensure_ntff_hook():
    """Make ``antenv.axon_hooks`` importable so trace-enabled runs work (or
    degrade gracefully). Profiling only; correctness never depends on it."""
    try:
        import antenv.axon_hooks  # noqa: F401
        return True
    except ImportError:
        pass
    import sys
    import types
    hook = None
    try:
        from trn_agent_boot.trn_boot import _ntff_profile_via_ctypes
        hook = _ntff_profile_via_ctypes("/opt/axon/libaxon_pjrt.so")
    except Exception:
        pass
    try:
        import antenv
        mod = types.ModuleType("antenv.axon_hooks")
        state = {"hook": hook}
        mod.get_axon_ntff_profile_hook = lambda: state["hook"]
        mod.set_axon_ntff_profile_hook = lambda h: state.update(hook=h)
        sys.modules["antenv.axon_hooks"] = mod
        antenv.axon_hooks = mod
    except Exception as e:  # pragma: no cover
        print(f"ntff hook setup failed ({e}); running without trace")
        return False
    return hook is not None


def build_nc():
    nc = bacc.Bacc(None)
    q_d = nc.dram_tensor("q4", [4 * DH, N], F16, kind="ExternalInput")
    k_d = nc.dram_tensor("k4", [4 * DH, N], F16, kind="ExternalInput")
    vt_d = nc.dram_tensor("vTo", [DIM, NCH, DH + 1], F16, kind="ExternalInput")
    o_d = nc.dram_tensor("o", [DH + 1, N], F16, kind="ExternalOutput")

    with tile.TileContext(nc) as tc:
        with (
            tc.tile_pool(name="singles", bufs=1) as singles,
            tc.tile_pool(name="ep", bufs=6) as ep,
            tc.tile_pool(name="outp", bufs=3) as outp,
            tc.tile_pool(name="psS", bufs=6, space="PSUM") as psS,
            tc.tile_pool(name="psO", bufs=1, space="PSUM") as psO,
            tc.tile_pool(name="psPad", bufs=1, space="PSUM") as psPad,
        ):
            q_sb = singles.tile([4 * DH, N], F16)
            k_sb = singles.tile([4 * DH, N], F16)
            vT = singles.tile([DIM, NCH, DH + 1], F16)
            pad_ps = psPad.tile([32, TI], F32)
            # k fully first (i-tile 0 needs every chunk), then q tile 0,
            # then v^T (needed LAG chunks into the main loop), then the rest.
            nc.sync.dma_start(out=k_sb[:, 0:TI], in_=k_d[:, 0:TI])
            nc.sync.dma_start(out=q_sb[:, 0:TI], in_=q_d[:, 0:TI])
            nc.sync.dma_start(out=k_sb[:, TI : N // 2], in_=k_d[:, TI : N // 2])
            nc.sync.dma_start(out=k_sb[:, N // 2 : N], in_=k_d[:, N // 2 : N])
            nc.sync.dma_start(out=vT[:], in_=vt_d[:])
            nc.sync.dma_start(out=q_sb[:, TI:N], in_=q_d[:, TI:N])

            def emit_epilogue(t, pO):
                if t == NT - 1:
                    # pure tail: two half-width chains so the first half's
                    # DMA runs under the second half's reduction.
                    for h in range(2):
                        sl = slice(h * (TI // 2), (h + 1) * (TI // 2))
                        tmp_h = outp.tile([DH + 1, TI // 2], F32,
                                          tag="tmpl%d" % h)
                        nc.scalar.copy(tmp_h[:],
                                       pO[bass.ds(64, DH + 1), sl])
                        os_h = outp.tile([DH + 1, TI // 2], F16,
                                         tag="osl%d" % h)
                        nc.vector.tensor_add(os_h[:], pO[0 : DH + 1, sl],
                                             tmp_h[:])
                        lo = t * TI + h * (TI // 2)
                        nc.sync.dma_start(out=o_d[:, lo : lo + TI // 2],
                                          in_=os_h[:])
                    return
                tmp = outp.tile([DH + 1, TI], F32, tag="tmp")
                nc.scalar.copy(tmp[:], pO[bass.ds(64, DH + 1), :])
                os33 = outp.tile([DH + 1, TI], F16, tag="os")
                nc.vector.tensor_add(os33[:], pO[0 : DH + 1, :], tmp[:])
                nc.sync.dma_start(out=o_d[:, bass.ts(t, TI)], in_=os33[:])

            # Continuous software pipeline over all NT*NCH chunks: the PV
            # stream lags the S^T/exp stream by LAG chunks GLOBALLY, so
            # every iteration issues [4 PVs][4 S^Ts][exps][pads].
            LAG = 12
            pOs = {}   # tile -> PSUM accumulator
            ess = {}   # tile -> list of per-iteration E tiles [128, 4, TI]

            def pv_mms(tp, cp):
                pO = pOs[tp]
                e4 = ess[tp][cp // 4]
                for j in range(4):
                    c = cp + j
                    half = pO[bass.ds(64 * (c % 2), DH + 1), :]
                    nc.tensor.matmul(half, vT[:, c, :], e4[:, j, :],
                                     start=(c < 2), stop=(c >= NCH - 2),
                                     skip_group_check=True)

            def st_mm(pS_ap, t, c):
                b = 32 * (c % 4)
                nc.tensor.matmul(pS_ap,
                                 k_sb[bass.ds(b, DH), bass.ts(c, CH)],
                                 q_sb[bass.ds(b, DH), bass.ts(t, TI)],
                                 start=True, stop=True, tile_position=(b, 0))

            def pad_mm(ncols):
                # dummy matmul into the dedicated scratch bank (never read);
                # fills PE idle so the HAM p-state never re-throttles.
                nc.tensor.matmul(pad_ps[:, 0:ncols], k_sb[0:DH, 0:DH],
                                 q_sb[0:DH, 0:ncols], start=True,
                                 stop=True, skip_group_check=True)

            NG = NT * NCH
            ep_due = None  # delayed one iteration so the last PVs complete
            for g in range(0, NG + LAG + 4, 4):
                if ep_due is not None:
                    emit_epilogue(ep_due, pOs.pop(ep_due))
                    ep_due = None
                # lagged PV stream (chunks g-LAG .. g-LAG+3)
                gp = g - LAG
                if gp >= 0 and gp < NG:
                    tp, cp = divmod(gp, NCH)
                    pv_mms(tp, cp)
                    if cp + 4 == NCH:
                        ep_due = tp
                if g >= NG:
                    continue
                # S^T + exp stream (chunks g .. g+3 of tile ti)
                ti, c0 = divmod(g, NCH)
                if c0 == 0:
                    pO_t = psO.tile([DIM, TI], F32, tag="po")
                    pOs[ti] = pO_t
                    ess[ti] = [None] * (NCH // 4)
                # PSUM: DVE double (bufs=2) + ACT pair (bufs=1); the four
                # S^T matmuls cover all four 32-row PE bands.
                pS_d = psS.tile([DIM, 2, TI], F32, tag="psd", bufs=2)
                pS_a = psS.tile([DIM, 2, TI], F32, tag="psa", bufs=1)
                st_mm(pS_d[:, 0, :], ti, c0)
                st_mm(pS_d[:, 1, :], ti, c0 + 1)
                st_mm(pS_a[:, 0, :], ti, c0 + 2)
                st_mm(pS_a[:, 1, :], ti, c0 + 3)
                e4 = ep.tile([DIM, 4, TI], F16, tag="e4")
                ess[ti][c0 // 4] = e4
                if c0 in (12, 24):
                    # rebalance: ACT takes the DVE tile's slot-1 chunk,
                    # emitted FIRST so the shared psum bank frees early.
                    nc.scalar.activation(e4[:, 1, :], pS_d[:, 1, :],
                                         func=EXP)
                    nc.vector.tensor_scalar(
                        out=e4[:, 0, :].bitcast(I16), in0=pS_d[:, 0, :],
                        scalar1=EXP_A, scalar2=EXP_B,
                        op0=mybir.AluOpType.mult, op1=mybir.AluOpType.add)
                else:
                    nc.vector.tensor_scalar(
                        out=e4[:, 0:2, :].bitcast(I16), in0=pS_d[:],
                        scalar1=EXP_A, scalar2=EXP_B,
                        op0=mybir.AluOpType.mult, op1=mybir.AluOpType.add)
                nc.scalar.activation(e4[:, 2:4, :], pS_a[:], func=EXP)
                # pads AFTER the exps in priority: the scheduler pops one
                # exactly when the next real matmul isn't ready yet.
                for _ in range(PADS_PER_ITER if g >= LAG else 6):
                    pad_mm(PAD_COLS)
    nc.compile()
    return nc


def kernel(input, w_qkv, w_out, b_out):
    global LAST_RESULTS
    input = np.asarray(input, dtype=np.float32)
    w_qkv = np.asarray(w_qkv, dtype=np.float32)
    w_out = np.asarray(w_out, dtype=np.float32)
    b_out = np.asarray(b_out, dtype=np.float32)

    b, c, X, Y, Z = input.shape
    n = X * Y * Z
    assert (b, c, n) == (2, DIM, N), (b, c, n)
    xf = input.reshape(b, c, n)
    scale = DH ** -0.5
    hid = HEADS * DH

    in_maps = []
    for core in range(N_CORES):
        bi, h = divmod(core, HEADS)
        wq = w_qkv[h * DH : (h + 1) * DH, :] * scale
        wk = w_qkv[hid + h * DH : hid + (h + 1) * DH, :]
        wv = w_qkv[2 * hid + h * DH : 2 * hid + (h + 1) * DH, :]
        xb = xf[bi]
        q = (wq @ xb).astype(_np_f16)            # [32, N]
        k = (wk @ xb).astype(_np_f16)
        v = (wv @ xb).astype(_np_f16)            # [32, N]
        vt = np.empty((DIM, NCH, DH + 1), _np_f16)
        vt[:, :, DH] = 1.0
        vt[:, :, 0:DH] = v.T.reshape(NCH, CH, DH).transpose(1, 0, 2)
        in_maps.append({
            "q4": np.ascontiguousarray(np.tile(q, (4, 1))),
            "k4": np.ascontiguousarray(np.tile(k, (4, 1))),
            "vTo": vt,
        })

    nc = build_nc()
    hook_ok = _ensure_ntff_hook()  # also guards env-driven BASS_TRACE runs
    LAST_RESULTS = run_bass_kernel_spmd(nc, in_maps, list(range(N_CORES)),
                                        trace=TRACE and hook_ok)
    results = LAST_RESULTS.results

    out = np.zeros((b, c, n), np.float32)
    for core in range(N_CORES):
        bi, h = divmod(core, HEADS)
        o33 = results[core]["o"].astype(np.float32)
        attn = o33[0:DH] / o33[DH : DH + 1]
        out[bi] += w_out[:, h * DH : (h + 1) * DH] @ attn
    out += b_out[None, :, None]
    return out.reshape(b, c, X, Y, Z)
